# revision 1
# baseline (speedup 1.0000x reference)
"""Trainium2 Bass kernel for nn_Chan_spaAtt (SE-gated conv block).

Key observation: the spatial self-attention branch in the reference is dead
code -- `gamma*attn_out + xo` is discarded and the output depends only on
xo = x * sigmoid(xl + xg) through the final 3x3 conv + BN + ReLU.

Computation per sample (C=64, H=W=64, N=4096), BN affines folded host-side:
  t1   = relu(W1 @ x + b1)            [16, N]
  sarg = W2 @ t1 + (b2 + xg)          [64, N]
  xg   = G2 @ relu(G1 @ mean(x) + bg1) + bg2   [64, 1]
  xo   = x * sigmoid(sarg)            [64, N]
  y    = relu(conv3x3(xo, CW) + cb)   [64, N]

Sharding: pure data parallelism, one sample per NeuronCore (B=8, 8 cores).
On-chip layout: channels on partitions (64), spatial flat on free dim.
conv3x3 = 9 shifted matmuls over a zero-padded [64, 66*66-ish] xo buffer.
"""

import sys

if "/opt/trn_rl_repo" not in sys.path:
    sys.path.insert(0, "/opt/trn_rl_repo")

import numpy as np

import concourse.bass as bass
import concourse.bacc as bacc
import concourse.mybir as mybir
import concourse.tile as tile
from concourse.bass_utils import run_bass_kernel_spmd

B, C, H, W = 8, 64, 64, 64
N = H * W
INTER = 16
EPS = 1e-5
PW = W + 2          # padded row stride = 66
HEAD = PW + 1       # zeros before pixel (0,0) = 67
PAD_LEN = HEAD + PW * (H - 1) + W + HEAD  # = 67 + 63*66 + 64 + 67 = 4356
PAD_ALLOC = PAD_LEN + 2  # +2 slack so slice-then-rearrange stays in bounds
CHUNK = 512
NCHUNK = N // CHUNK  # 8
ROWS_PER_CHUNK = CHUNK // W  # 8

F32 = mybir.dt.float32
F32R = mybir.dt.float32r

# weights-blob column layout (f32r, 64 partitions)
O_W1T = 0
O_W2T = 16          # partitions 0:16
O_GW1T = 80
O_GW2T = 96         # partitions 0:16
O_CWT = 160
O_B1 = 736          # partitions 0:16
O_GB1 = 737         # partitions 0:16
O_BSIG = 738
O_CB = 739
WCOLS = 740
XPART = 1024        # x arrives in 4 quarters; quarter 0 rides in the blob DMA

_prog_cache = {}


def _pix(r, w):
    """Flat index of valid pixel (r, w) in the padded xo buffer."""
    return HEAD + r * PW + w


def build_program(n_cores=8):
    nc = bacc.Bacc("TRN2", debug=False, target_bir_lowering=False,
                   num_devices=n_cores)

    blob_d = nc.dram_tensor("blob", [C, WCOLS + XPART], F32R,
                            kind="ExternalInput").ap()
    xrest_d = nc.dram_tensor("xrest", [C, N - XPART], F32R,
                             kind="ExternalInput").ap()
    cwp_d = nc.dram_tensor("cwp", [2 * C, 3 * C], F32R,
                           kind="ExternalInput").ap()
    y_d = nc.dram_tensor("y", [C, N], F32, kind="ExternalOutput").ap()

    with tile.TileContext(nc) as tc:
        with tc.tile_pool(name="big", bufs=1) as bpool, \
             tc.tile_pool(name="work", bufs=3) as wpool, \
             tc.tile_pool(name="ps1p", bufs=2, space="PSUM") as pp1, \
             tc.tile_pool(name="ps2p", bufs=2, space="PSUM") as pp2, \
             tc.tile_pool(name="psyp", bufs=2, space="PSUM") as ppy:

            big = bpool.tile([C, WCOLS + N], F32R, tag="big")
            # DMA 1: weights + first x quarter -- a single semaphore gates
            # the first matmul (walrus allows only one sync wait per matmul).
            nc.sync.dma_start(big[:, 0:WCOLS + XPART], blob_d)
            for q in range(1, N // XPART):
                nc.sync.dma_start(
                    big[:, WCOLS + q * XPART: WCOLS + (q + 1) * XPART],
                    xrest_d[:, (q - 1) * XPART: q * XPART])

            w1t = big[:, O_W1T:O_W1T + INTER]
            w2t = big[0:INTER, O_W2T:O_W2T + C]
            gw1t = big[:, O_GW1T:O_GW1T + INTER]
            gw2t = big[0:INTER, O_GW2T:O_GW2T + C]
            cwt = big[:, O_CWT:O_CWT + 9 * C]
            b1 = big[0:INTER, O_B1:O_B1 + 1].bitcast(F32)
            gb1 = big[0:INTER, O_GB1:O_GB1 + 1].bitcast(F32)
            bsig = big[:, O_BSIG:O_BSIG + 1].bitcast(F32)
            cb = big[:, O_CB:O_CB + 1].bitcast(F32)
            x_sb = big[:, WCOLS:WCOLS + N]

            cwp = bpool.tile([2 * C, 3 * C], F32R, tag="cwp")
            nc.sync.dma_start(cwp[:], cwp_d)

            # ---- padded xo buffer; zero the halo regions ----
            # partitions 0:64 hold xo_pad (copy A); partitions 64:128 hold the
            # same data shifted left by 2*PW (copy B) so one K=128 matmul sums
            # the dy=-1 (A) and dy=+1 (B) conv taps at a single rhs offset.
            xo_pad = bpool.tile([2 * C, PAD_ALLOC], F32R, tag="xopad")
            nc.vector.memset(xo_pad[:].bitcast(mybir.dt.uint32), 0)

            # ---- mm1 + relu, chunk 0 first (PE head: observes blob DMA) ----
            mm1_insts, mm2_insts = [], []
            t1s = {}

            def emit_mm1(ci):
                xc = x_sb[:, ci * CHUNK:(ci + 1) * CHUNK]
                ps1 = pp1.tile([INTER, CHUNK], F32, tag="ps1")
                mm1_insts.append(nc.tensor.matmul(ps1[:], w1t, xc,
                                                  start=True, stop=True))
                t1 = wpool.tile([INTER, CHUNK], F32R, tag="t1")
                nc.scalar.activation(t1[:], ps1[:],
                                     mybir.ActivationFunctionType.Relu,
                                     bias=b1)
                t1s[ci] = t1

            emit_mm1(0)

            # ---- global branch: xg folded into per-channel sigmoid bias D ----
            g_parts = wpool.tile([C, 4], F32, tag="gparts")
            for q in range(4):
                nc.vector.reduce_sum(
                    g_parts[:, q:q + 1],
                    x_sb.bitcast(F32)[:, q * XPART:(q + 1) * XPART],
                    axis=mybir.AxisListType.X)
            g_raw = wpool.tile([C, 1], F32, tag="graw")
            nc.vector.reduce_sum(g_raw[:], g_parts[:],
                                 axis=mybir.AxisListType.X)
            ps_g1 = pp1.tile([INTER, 1], F32, tag="ps1")
            gmm1 = nc.tensor.matmul(ps_g1[:], gw1t.bitcast(F32), g_raw[:],
                                    start=True, stop=True)
            bass._add_dep_helper(gmm1.ins, mm1_insts[0].ins, sync=False,
                                 reason="PE observes blob DMA via mm1[0]")
            g1 = wpool.tile([INTER, 1], F32, tag="g1")
            nc.scalar.activation(g1[:], ps_g1[:],
                                 mybir.ActivationFunctionType.Relu,
                                 bias=gb1, scale=1.0 / N)
            ps_g2 = pp2.tile([C, 1], F32, tag="ps2")
            gmm2 = nc.tensor.matmul(ps_g2[:], gw2t.bitcast(F32), g1[:],
                                    start=True, stop=True)
            dbias = wpool.tile([C, 1], F32, tag="dbias")
            nc.scalar.activation(dbias[:], ps_g2[:],
                                 mybir.ActivationFunctionType.Identity,
                                 bias=bsig)

            # ---- phase 1: SE gating, chunked over spatial ----
            for ci in range(NCHUNK):
                if ci > 0:
                    emit_mm1(ci)
                xc = x_sb[:, ci * CHUNK:(ci + 1) * CHUNK]
                ps2 = pp2.tile([C, CHUNK], F32, tag="ps2")
                mm2_insts.append(nc.tensor.matmul(ps2[:], w2t, t1s.pop(ci)[:],
                                                  start=True, stop=True))
                if ci >= 2:
                    bass._add_dep_helper(
                        mm1_insts[ci].ins, mm2_insts[ci - 2].ins, sync=False,
                        reason="pipeline order: ps1 slot release observed")
                sig = wpool.tile([C, CHUNK], F32, tag="sig")
                nc.scalar.activation(sig[:], ps2[:],
                                     mybir.ActivationFunctionType.Sigmoid,
                                     bias=dbias[:])
                r0 = ci * ROWS_PER_CHUNK
                dst = xo_pad[0:C, _pix(r0, 0): _pix(r0, 0) + ROWS_PER_CHUNK * PW]
                dst = dst.rearrange("p (r w) -> p r w", w=PW)[:, :, 0:W]
                xcr = xc.bitcast(F32).rearrange("p (r w) -> p r w", w=W)
                sigr = sig[:].rearrange("p (r w) -> p r w", w=W)
                nc.vector.tensor_mul(dst, xcr, sigr)
                s0 = max(2 * PW, _pix(r0, 0))
                e0 = _pix(r0 + ROWS_PER_CHUNK - 1, W)
                nc.sync.dma_start(xo_pad[C:2 * C, s0 - 2 * PW:e0 - 2 * PW],
                                  xo_pad[0:C, s0:e0])

            # ---- phase 2: 3x3 conv as 3 paired + 3 single matmuls ----
            def shifted_rhs(parts, o):
                rhs = xo_pad[0:parts, o: o + ROWS_PER_CHUNK * PW]
                return rhs.rearrange("p (r w) -> p r w", w=PW)[:, :, 0:W]

            for cj in range(NCHUNK // 2):
                psy = ppy.tile([C, 2 * CHUNK], F32, tag="psy")
                for h in range(2):
                    r0 = (2 * cj + h) * ROWS_PER_CHUNK
                    half = psy[:, h * CHUNK:(h + 1) * CHUNK]
                    for j, dx in enumerate((-1, 0, 1)):
                        nc.tensor.matmul(half, cwp[:, j * C:(j + 1) * C],
                                         shifted_rhs(2 * C, _pix(r0 - 1, dx)),
                                         start=(j == 0), stop=False)
                    for j, dx in enumerate((-1, 0, 1)):
                        k = 3 + (dx + 1)
                        nc.tensor.matmul(half, cwt[:, k * C:(k + 1) * C],
                                         shifted_rhs(C, _pix(r0, dx)),
                                         start=False, stop=(j == 2))
                ybuf = wpool.tile([C, 2 * CHUNK], F32, tag="ybuf")
                nc.scalar.activation(ybuf[:], psy[:],
                                     mybir.ActivationFunctionType.Relu,
                                     bias=cb)
                nc.sync.dma_start(
                    y_d[:, 2 * cj * CHUNK:(2 * cj + 2) * CHUNK], ybuf[:])

    nc.compile()
    return nc


def _affine(s, b, m, v):
    inv = s / np.sqrt(v + EPS)
    return inv, b - m * inv


def prepare_weights(inputs):
    f = lambda k: np.asarray(inputs[k], dtype=np.float32)
    a1, c1 = _affine(f("ls1"), f("lbb1"), f("lm1"), f("lv1"))
    W1 = a1[:, None] * f("lw1")
    B1 = a1 * f("lb1") + c1
    a2, c2 = _affine(f("ls2"), f("lbb2"), f("lm2"), f("lv2"))
    W2 = a2[:, None] * f("lw2")
    B2 = a2 * f("lb2") + c2
    ag1, cg1 = _affine(f("gs1"), f("gbb1"), f("gm1"), f("gv1"))
    G1 = ag1[:, None] * f("gw1")
    Bg1 = ag1 * f("gb1") + cg1
    ag2, cg2 = _affine(f("gs2"), f("gbb2"), f("gm2"), f("gv2"))
    G2 = ag2[:, None] * f("gw2")
    Bg2 = ag2 * f("gb2") + cg2
    ac, cc = _affine(f("cs"), f("cbb"), f("cm"), f("cv"))
    CW = ac[:, None, None, None] * f("cw")        # [O, C, 3, 3]
    CB = ac * f("cb") + cc
    cwt = np.ascontiguousarray(
        CW.transpose(1, 2, 3, 0).reshape(C, 9 * C))  # [c, (ky kx) o]
    col = lambda v: np.ascontiguousarray(v.reshape(-1, 1), dtype=np.float32)
    cn = lambda v: np.ascontiguousarray(v, dtype=np.float32)
    return {
        "w1t": cn(W1.T), "b1": col(B1),
        "w2t": cn(W2.T),
        "gw1t": cn(G1.T), "gb1": col(Bg1),
        "gw2t": cn(G2.T), "bsig": col(B2 + Bg2),
        "cwt": cn(cwt), "cb": col(CB),
    }


def assemble_wblob(shared):
    wb = np.zeros((C, WCOLS), np.float32)
    wb[:, O_W1T:O_W1T + INTER] = shared["w1t"]
    wb[0:INTER, O_W2T:O_W2T + C] = shared["w2t"]
    wb[:, O_GW1T:O_GW1T + INTER] = shared["gw1t"]
    wb[0:INTER, O_GW2T:O_GW2T + C] = shared["gw2t"]
    wb[:, O_CWT:O_CWT + 9 * C] = shared["cwt"]
    wb[0:INTER, O_B1] = shared["b1"][:, 0]
    wb[0:INTER, O_GB1] = shared["gb1"][:, 0]
    wb[:, O_BSIG] = shared["bsig"][:, 0]
    wb[:, O_CB] = shared["cb"][:, 0]
    return wb


def assemble_cwp(shared):
    # cwt[c, (ky*3+kx)*64 + o]; pairs stack ky=0 on top, ky=2 below, per kx
    cwt = shared["cwt"]
    cwp = np.zeros((2 * C, 3 * C), np.float32)
    for j in range(3):
        cwp[0:C, j * C:(j + 1) * C] = cwt[:, (0 + j) * C:(0 + j + 1) * C]
        cwp[C:2 * C, j * C:(j + 1) * C] = cwt[:, (6 + j) * C:(6 + j + 1) * C]
    return cwp


def make_core_inputs(inputs):
    shared = prepare_weights(inputs)
    wb = assemble_wblob(shared)
    cwp = np.ascontiguousarray(assemble_cwp(shared))
    x = np.asarray(inputs["x"], dtype=np.float32)
    maps = []
    for i in range(B):
        xi = x[i].reshape(C, N)
        maps.append({
            "blob": np.ascontiguousarray(
                np.concatenate([wb, xi[:, :XPART]], axis=1)),
            "xrest": np.ascontiguousarray(xi[:, XPART:]),
            "cwp": cwp,
        })
    return maps


def _run(inputs, trace=False):
    in_maps = make_core_inputs(inputs)
    if "prog" not in _prog_cache:
        _prog_cache["prog"] = build_program(B)
    nc = _prog_cache["prog"]
    res = run_bass_kernel_spmd(nc, in_maps, list(range(B)), trace=trace)
    out = np.stack([r["y"].reshape(C, H, W) for r in res.results])
    return out.astype(np.float32), res


def kernel(**inputs):
    out, _ = _run(inputs, trace=False)
    return out


def kernel_traced(inputs):
    return _run(inputs, trace=True)


def reference_numpy(inputs):
    """Pure-numpy emulation of the (dead-code-eliminated) reference, using the
    same folded weights as the device kernel. For algebra validation only."""
    shared = prepare_weights(inputs)
    x = np.asarray(inputs["x"], dtype=np.float32)  # [B, C, H, W]
    f = lambda k: np.asarray(inputs[k], dtype=np.float32)
    a1, c1 = _affine(f("ls1"), f("lbb1"), f("lm1"), f("lv1"))
    B1 = a1 * f("lb1") + c1
    out = np.empty_like(x)
    for i in range(B):
        xs = x[i].reshape(C, N)
        t1 = np.maximum(shared["w1t"].T @ xs + B1[:, None], 0.0)
        g = xs.mean(axis=1, keepdims=True)
        g1 = np.maximum(shared["gw1t"].T @ g + shared["gb1"], 0.0)
        d = shared["gw2t"].T @ g1 + shared["bsig"]
        sarg = shared["w2t"].T @ t1 + d
        xo = xs * (1.0 / (1.0 + np.exp(-sarg)))
        xop = np.zeros((C, H + 2, W + 2), np.float32)
        xop[:, 1:-1, 1:-1] = xo.reshape(C, H, W)
        y = np.zeros((C, N), np.float32)
        for k in range(9):
            ky, kx = divmod(k, 3)
            sh = xop[:, ky:ky + H, kx:kx + W].reshape(C, N)
            y += shared["cwt"][:, k * C:(k + 1) * C].T @ sh
        y = np.maximum(y + shared["cb"], 0.0)
        out[i] = y.reshape(C, H, W)
    return out



# revision 3
# speedup vs baseline: 1.3013x; 1.3013x over previous
"""Trainium2 Bass kernel for nn_Chan_spaAtt (SE-gated conv block), v2.

The spatial self-attention branch in the reference is dead code -- the output
depends only on xo = x * sigmoid(xl + xg) through the final 3x3 conv + BN +
ReLU (all BN affines folded host-side):

  t1   = relu(W1 @ x + b1)                      [16, N]
  d    = G2 @ relu(G1 @ mean(x) + bg1) + bsig   [64, 1]
  sarg = W2 @ t1                                [64, N]
  xo   = x * sigmoid(sarg + d)                  [64, N]
  y    = relu(conv3x3(xo, CW) + cb)             [64, N]

Sharding: one sample per NeuronCore (B=8).

v2 layout: everything bf16 on-chip, 128 partitions everywhere.
 - x_dual [128, 4096]: partition c+64s holds x[c, row+s] per 8-row chunk
   (dual row-shift).  SE phase computes each pixel twice (once per shift)
   at zero extra cost: engine time scales with the free dim only.
 - xo_pad [128, 40*132]: copy A (partitions 0:64) = padded xo grid with
   row stride 66; copy B (64:128) holds the next row's values at the same
   column (written directly by the dual-layout SE multiply).
 - conv3x3 = 6 dense K=128 matmuls per 16-row tile: M=128 packs (out
   channel x output-row-parity), K=128 packs (in channel x row shift).
   12288 PE rows total vs 24576 in the 9-tap formulation.
 - global-branch mean via DVE reduce over a [128, 2048] half-stacked copy
   of x (halves the reduce free size); stacked-G1 matmul recombines the
   partition halves exactly.
"""

import sys

if "/opt/trn_rl_repo" not in sys.path:
    sys.path.insert(0, "/opt/trn_rl_repo")

import numpy as np
import ml_dtypes

import concourse.bass as bass
import concourse.bacc as bacc
import concourse.mybir as mybir
import concourse.tile as tile
from concourse.bass_utils import run_bass_kernel_spmd

B, C, H, W = 8, 64, 64, 64
N = H * W
C2 = 2 * C          # 128
INTER = 16
EPS = 1e-5
PW = W + 2          # padded row stride = 66
BW = 2 * PW         # conv-view block width = 132 (one row pair)
NBLK = 40           # blocks in xo_pad; 40*132 = 5280 columns
PADC = NBLK * BW
HEAD = PW + 1       # flat offset of grid pixel (0, 0) = 67
CHUNK = 512
NCHUNK = N // CHUNK          # 8
ROWS_PER_CHUNK = CHUNK // W  # 8

TAPS = ((-1, -1), (-1, 0), (-1, 1), (1, -1), (1, 0), (1, 1))

F32 = mybir.dt.float32
BF16 = mybir.dt.bfloat16
AF = mybir.ActivationFunctionType
ALU = mybir.AluOpType
BFNP = ml_dtypes.bfloat16

# weight blob (bf16, 128 partitions) column layout
O_W1P = 0     # [128, 32]
O_W2P = 32    # [32, 128] on partitions 0:32
O_GW1P = 160  # [128, 16]
O_GW2P = 176  # [16, 128] on partitions 0:16
WCOLS = 304
# f32 per-partition bias columns
FB_B1 = 0     # rows 0:32
FB_BSIG = 1   # rows 0:128
FB_CB = 2     # rows 0:128
FB_GB1 = 3    # rows 0:16
FB_ZERO = 4   # dummy-sigmoid source
FBCOLS = 5

_prog_cache = {}


def _pix(r, w):
    """Flat column of valid grid pixel (r, w) in xo_pad copy A."""
    return HEAD + r * PW + w


def build_program(n_cores=8):
    nc = bacc.Bacc("TRN2", debug=False, target_bir_lowering=False,
                   num_devices=n_cores)

    fb_d = nc.dram_tensor("fb", [C2, FBCOLS], F32, kind="ExternalInput").ap()
    wsm_d = nc.dram_tensor("wsm", [C2, WCOLS], BF16, kind="ExternalInput").ap()
    xh_d = nc.dram_tensor("xh", [C2, N // 2], BF16, kind="ExternalInput").ap()
    xd_d = nc.dram_tensor("xd", [C2, N], BF16, kind="ExternalInput").ap()
    cw6_d = nc.dram_tensor("cw6", [C2, 6 * C2], BF16,
                           kind="ExternalInput").ap()
    y_d = nc.dram_tensor("y", [C, H // 2, C2], BF16, kind="ExternalOutput").ap()

    with tile.TileContext(nc) as tc:
        with tc.tile_pool(name="big", bufs=1) as bpool, \
             tc.tile_pool(name="work", bufs=3) as wpool, \
             tc.tile_pool(name="t1s", bufs=8) as tpool, \
             tc.tile_pool(name="ps1p", bufs=2, space="PSUM") as pp1, \
             tc.tile_pool(name="ps2p", bufs=4, space="PSUM") as pp2, \
             tc.tile_pool(name="psyp", bufs=2, space="PSUM") as ppy:

            fb = bpool.tile([C2, FBCOLS], F32, tag="fb")
            nc.sync.dma_start(fb[:], fb_d)
            # dummy sigmoid: forces the one table set (sigmoid_and_others,
            # which also holds relu + identity) to load at t~0 on ACT.
            scr = bpool.tile([1, 1], F32, tag="scr")
            nc.scalar.activation(scr[:], fb[0:1, FB_ZERO:FB_ZERO + 1],
                                 AF.Sigmoid)

            wsm = bpool.tile([C2, WCOLS], BF16, tag="wsm")
            nc.sync.dma_start(wsm[:], wsm_d)
            xh = bpool.tile([C2, N // 2], BF16, tag="xh")
            for q in range(4):
                nc.sync.dma_start(xh[:, q * 512:(q + 1) * 512],
                                  xh_d[:, q * 512:(q + 1) * 512])
            xd = bpool.tile([C2, N], BF16, tag="xd")
            for q in range(NCHUNK):
                nc.sync.dma_start(xd[:, q * CHUNK:(q + 1) * CHUNK],
                                  xd_d[:, q * CHUNK:(q + 1) * CHUNK])
            cw6 = bpool.tile([C2, 6 * C2], BF16, tag="cw6")
            nc.sync.dma_start(cw6[:], cw6_d)

            w1p = wsm[:, O_W1P:O_W1P + 32]
            w2p = wsm[0:32, O_W2P:O_W2P + C2]
            gw1p = wsm[:, O_GW1P:O_GW1P + INTER]
            gw2p = wsm[0:INTER, O_GW2P:O_GW2P + C2]
            b1 = fb[0:32, FB_B1:FB_B1 + 1]
            bsig = fb[:, FB_BSIG:FB_BSIG + 1]
            cb = fb[:, FB_CB:FB_CB + 1]
            gb1 = fb[0:INTER, FB_GB1:FB_GB1 + 1]

            # ---- xo_pad halo memsets (gpsimd; off every critical path) ----
            xo_pad = bpool.tile([C2, PADC], BF16, tag="xopad")
            nc.gpsimd.memset(xo_pad[:, 0:HEAD], 0)
            gaps = xo_pad[:, HEAD + W:HEAD + W + H * PW]
            gaps = gaps.rearrange("p (r w) -> p r w", w=PW)[:, :, 0:2]
            nc.gpsimd.memset(gaps, 0)
            nc.gpsimd.memset(xo_pad[:, _pix(H - 1, W) + 2:PADC], 0)
            # copy B's slot for grid row 64 (the bottom halo) stays zero
            nc.gpsimd.memset(xo_pad[C:C2, _pix(H - 1, 0):_pix(H - 1, W)], 0)

            # ---- global mean partials on DVE (from the half-stacked copy) --
            gparts = wpool.tile([C2, 4], F32, tag="gparts")
            for q in range(4):
                nc.vector.reduce_sum(gparts[:, q:q + 1],
                                     xh[:, q * 512:(q + 1) * 512],
                                     axis=mybir.AxisListType.X)
            g128 = wpool.tile([C2, 1], F32, tag="g128")
            nc.vector.reduce_sum(g128[:], gparts[:],
                                 axis=mybir.AxisListType.X)
            g128b = wpool.tile([C2, 1], BF16, tag="g128b")
            nc.vector.tensor_copy(g128b[:], g128[:])

            # ---- SE phase 1: mm1 + t1 relu for every chunk ----
            t1s = {}
            mm1_insts = []

            def emit_mm1(ci):
                ps1 = pp1.tile([32, CHUNK], F32, tag="ps1")
                mm1_insts.append(nc.tensor.matmul(
                    ps1[:], w1p, xd[:, ci * CHUNK:(ci + 1) * CHUNK],
                    start=True, stop=True))
                t1 = tpool.tile([32, CHUNK], BF16, tag="t1")
                if ci < 5:
                    nc.scalar.activation(t1[:], ps1[:], AF.Relu, bias=b1)
                else:
                    nc.vector.tensor_scalar(t1[:], ps1[:], b1, 0.0,
                                            ALU.add, ALU.max)
                t1s[ci] = t1

            emit_mm1(0)
            emit_mm1(1)
            emit_mm1(2)
            emit_mm1(3)

            # ---- global branch MLP (PE ops land after mm1_3 in queue) ----
            psg1 = pp1.tile([INTER, 1], F32, tag="ps1")
            nc.tensor.matmul(psg1[:], gw1p, g128b[:], start=True, stop=True)
            g1 = wpool.tile([INTER, 1], BF16, tag="g1")
            nc.scalar.activation(g1[:], psg1[:], AF.Relu, bias=gb1,
                                 scale=1.0 / N)
            psg2 = pp2.tile([C2, 1], F32, tag="ps2")
            nc.tensor.matmul(psg2[:], gw2p, g1[:], start=True, stop=True)
            dbias = wpool.tile([C2, 1], F32, tag="dbias")
            nc.scalar.activation(dbias[:], psg2[:], AF.Identity, bias=bsig)

            for ci in range(4, NCHUNK):
                emit_mm1(ci)

            # ---- SE phase 2: mm2 + sigmoid + xo multiply ----
            def emit_mul(ci):
                sig = wpool.tile([C2, CHUNK], BF16, tag="sig")
                nc.scalar.activation(sig[:], ps2s[ci][:], AF.Sigmoid,
                                     bias=dbias[:])
                r0 = 8 * ci
                nrow = 8 if ci < NCHUNK - 1 else 7
                ncol = nrow * W
                dst = xo_pad[:, _pix(r0, 0):_pix(r0, 0) + nrow * PW]
                dst = dst.rearrange("p (r w) -> p r w", w=PW)[:, :, 0:W]
                xcr = xd[:, ci * CHUNK:ci * CHUNK + ncol]
                xcr = xcr.rearrange("p (r w) -> p r w", w=W)
                sgr = sig[:, 0:ncol].rearrange("p (r w) -> p r w", w=W)
                nc.vector.tensor_mul(dst, xcr, sgr)
                if ci == NCHUNK - 1:
                    # last row of the image: top half only (copy B's slot for
                    # it is the row-64 halo, which must stay zero)
                    dst2 = xo_pad[0:C, _pix(r0 + 7, 0):_pix(r0 + 7, 0) + W]
                    nc.vector.tensor_mul(
                        dst2, xd[0:C, ci * CHUNK + ncol:(ci + 1) * CHUNK],
                        sig[0:C, ncol:CHUNK])

            ps2s = {}
            for ci in range(NCHUNK):
                ps2 = pp2.tile([C2, CHUNK], F32, tag="ps2")
                nc.tensor.matmul(ps2[:], w2p, t1s[ci][:],
                                 start=True, stop=True)
                ps2s[ci] = ps2
                emit_mul(ci)

            # copy B's column for grid row -1 must hold xo row 0
            nc.sync.dma_start(xo_pad[C:C2, _pix(-1, 0):_pix(-1, 0) + W],
                              xo_pad[0:C, _pix(0, 0):_pix(0, 0) + W])

            # ---- conv3x3: 6 dense 128x128 matmuls per 16-row tile ----
            xor_v = xo_pad[:].rearrange("p (t w) -> p t w", w=BW)
            for k in range(4):
                r0 = 16 * k
                psy = ppy.tile([C2, CHUNK], F32, tag="psy")
                for jj, (dlt, dx) in enumerate(TAPS):
                    t0 = (r0 + dlt + 1) // 2
                    rhs = xor_v[:, t0:t0 + 8, 1 + dx:1 + dx + W]
                    nc.tensor.matmul(psy[:], cw6[:, jj * C2:(jj + 1) * C2],
                                     rhs, start=(jj == 0), stop=(jj == 5))
                ybuf = wpool.tile([C2, CHUNK], BF16, tag="ybuf")
                nc.gpsimd.tensor_scalar(ybuf[:], psy[:], cb, 0.0,
                                        ALU.add, ALU.max)
                yb_r = ybuf[:].rearrange("p (t w) -> p t w", w=W)
                nc.sync.dma_start(y_d[:, 8 * k:8 * k + 8, 0:W], yb_r[0:C])
                nc.sync.dma_start(y_d[:, 8 * k:8 * k + 8, W:C2], yb_r[C:C2])

    nc.compile()
    return nc


def _affine(s, b, m, v):
    inv = s / np.sqrt(v + EPS)
    return inv, b - m * inv


def prepare_weights(inputs):
    f = lambda k: np.asarray(inputs[k], dtype=np.float32)
    a1, c1 = _affine(f("ls1"), f("lbb1"), f("lm1"), f("lv1"))
    W1 = a1[:, None] * f("lw1")              # [16, 64]
    B1 = a1 * f("lb1") + c1
    a2, c2 = _affine(f("ls2"), f("lbb2"), f("lm2"), f("lv2"))
    W2 = a2[:, None] * f("lw2")              # [64, 16]
    B2 = a2 * f("lb2") + c2
    ag1, cg1 = _affine(f("gs1"), f("gbb1"), f("gm1"), f("gv1"))
    G1 = ag1[:, None] * f("gw1")             # [16, 64]
    Bg1 = ag1 * f("gb1") + cg1
    ag2, cg2 = _affine(f("gs2"), f("gbb2"), f("gm2"), f("gv2"))
    G2 = ag2[:, None] * f("gw2")             # [64, 16]
    Bg2 = ag2 * f("gb2") + cg2
    ac, cc = _affine(f("cs"), f("cbb"), f("cm"), f("cv"))
    CW = ac[:, None, None, None] * f("cw")   # [64, 64, 3, 3] (o, c, ky, kx)
    CB = ac * f("cb") + cc

    w1p = np.zeros((C2, 32), np.float32)
    w1p[0:C, 0:INTER] = W1.T
    w1p[C:C2, INTER:32] = W1.T
    w2p = np.zeros((32, C2), np.float32)
    w2p[0:INTER, 0:C] = W2.T
    w2p[INTER:32, C:C2] = W2.T
    gw1p = np.concatenate([G1.T, G1.T], axis=0)        # [128, 16]
    gw2p = np.concatenate([G2.T, G2.T], axis=1)        # [16, 128]

    cw6 = np.zeros((C2, 6, C2), np.float32)
    for jj, (dlt, dx) in enumerate(TAPS):
        for s in (0, 1):
            for p in (0, 1):
                ky = dlt + s + 1 - p
                if 0 <= ky <= 2:
                    cw6[C * s:C * s + C, jj, C * p:C * p + C] = \
                        CW[:, :, ky, dx + 1].T

    wsm = np.zeros((C2, WCOLS), np.float32)
    wsm[:, O_W1P:O_W1P + 32] = w1p
    wsm[0:32, O_W2P:O_W2P + C2] = w2p
    wsm[:, O_GW1P:O_GW1P + INTER] = gw1p
    wsm[0:INTER, O_GW2P:O_GW2P + C2] = gw2p

    fb = np.zeros((C2, FBCOLS), np.float32)
    fb[0:32, FB_B1] = np.concatenate([B1, B1])
    fb[:, FB_BSIG] = np.concatenate([B2 + Bg2, B2 + Bg2])
    fb[:, FB_CB] = np.concatenate([CB, CB])
    fb[0:INTER, FB_GB1] = Bg1
    return {
        "wsm": wsm.astype(BFNP),
        "cw6": np.ascontiguousarray(cw6.reshape(C2, 6 * C2)).astype(BFNP),
        "fb": fb,
    }


def make_core_inputs(inputs):
    shared = prepare_weights(inputs)
    x = np.asarray(inputs["x"], dtype=np.float32)
    maps = []
    for i in range(B):
        xi = x[i]                                   # [64, 64, 64]
        xh = np.concatenate([xi[:, 0:H // 2, :].reshape(C, N // 2),
                             xi[:, H // 2:H, :].reshape(C, N // 2)], axis=0)
        xsh = np.concatenate([xi[:, 1:, :],
                              np.zeros((C, 1, W), np.float32)], axis=1)
        xd = np.concatenate([xi.reshape(C, N), xsh.reshape(C, N)], axis=0)
        maps.append({
            "fb": shared["fb"],
            "wsm": shared["wsm"],
            "cw6": shared["cw6"],
            "xh": np.ascontiguousarray(xh).astype(BFNP),
            "xd": np.ascontiguousarray(xd).astype(BFNP),
        })
    return maps


def _unpack_y(y3):
    # y3 [64, 32, 128] bf16: col block 0:64 = even rows, 64:128 = odd rows
    y3 = np.asarray(y3, dtype=np.float32)
    out = np.empty((C, H, W), np.float32)
    out[:, 0::2, :] = y3[:, :, 0:W]
    out[:, 1::2, :] = y3[:, :, W:C2]
    return out


def _run(inputs, trace=False):
    in_maps = make_core_inputs(inputs)
    if "prog" not in _prog_cache:
        _prog_cache["prog"] = build_program(B)
    nc = _prog_cache["prog"]
    res = run_bass_kernel_spmd(nc, in_maps, list(range(B)), trace=trace)
    out = np.stack([_unpack_y(r["y"]) for r in res.results])
    return out.astype(np.float32), res


def kernel(**inputs):
    out, _ = _run(inputs, trace=False)
    return out


def kernel_traced(inputs):
    return _run(inputs, trace=True)


def reference_numpy(inputs):
    """Pure-numpy emulation of the (dead-code-eliminated) reference using the
    same folded weights (f32, no bf16 rounding). Algebra validation only."""
    f = lambda k: np.asarray(inputs[k], dtype=np.float32)
    a1, c1 = _affine(f("ls1"), f("lbb1"), f("lm1"), f("lv1"))
    W1 = a1[:, None] * f("lw1")
    B1 = a1 * f("lb1") + c1
    a2, c2 = _affine(f("ls2"), f("lbb2"), f("lm2"), f("lv2"))
    W2 = a2[:, None] * f("lw2")
    B2 = a2 * f("lb2") + c2
    ag1, cg1 = _affine(f("gs1"), f("gbb1"), f("gm1"), f("gv1"))
    G1 = ag1[:, None] * f("gw1")
    Bg1 = ag1 * f("gb1") + cg1
    ag2, cg2 = _affine(f("gs2"), f("gbb2"), f("gm2"), f("gv2"))
    G2 = ag2[:, None] * f("gw2")
    Bg2 = ag2 * f("gb2") + cg2
    ac, cc = _affine(f("cs"), f("cbb"), f("cm"), f("cv"))
    CW = ac[:, None, None, None] * f("cw")
    CB = ac * f("cb") + cc
    x = np.asarray(inputs["x"], dtype=np.float32)
    out = np.empty_like(x)
    for i in range(B):
        xs = x[i].reshape(C, N)
        t1 = np.maximum(W1 @ xs + B1[:, None], 0.0)
        g = xs.mean(axis=1, keepdims=True)
        d = G2 @ np.maximum(G1 @ g + Bg1[:, None], 0.0) + (B2 + Bg2)[:, None]
        xo = xs / (1.0 + np.exp(-(W2 @ t1 + d)))
        xop = np.zeros((C, H + 2, W + 2), np.float32)
        xop[:, 1:-1, 1:-1] = xo.reshape(C, H, W)
        y = np.zeros((C, N), np.float32)
        for kk in range(9):
            ky, kx = divmod(kk, 3)
            sh = xop[:, ky:ky + H, kx:kx + W].reshape(C, N)
            y += CW[:, :, ky, kx] @ sh
        out[i] = np.maximum(y + CB[:, None], 0.0).reshape(C, H, W)
    return out


# revision 4
# speedup vs baseline: 1.5693x; 1.2060x over previous
"""Trainium2 Bass kernel for nn_Chan_spaAtt (SE-gated conv block), v2.

The spatial self-attention branch in the reference is dead code -- the output
depends only on xo = x * sigmoid(xl + xg) through the final 3x3 conv + BN +
ReLU (all BN affines folded host-side):

  t1   = relu(W1 @ x + b1)                      [16, N]
  d    = G2 @ relu(G1 @ mean(x) + bg1) + bsig   [64, 1]
  sarg = W2 @ t1                                [64, N]
  xo   = x * sigmoid(sarg + d)                  [64, N]
  y    = relu(conv3x3(xo, CW) + cb)             [64, N]

Sharding: one sample per NeuronCore (B=8).

v2 layout: everything bf16 on-chip, 128 partitions everywhere.
 - x_dual [128, 4096]: partition c+64s holds x[c, row+s] per 8-row chunk
   (dual row-shift).  SE phase computes each pixel twice (once per shift)
   at zero extra cost: engine time scales with the free dim only.
 - xo_pad [128, 40*132]: copy A (partitions 0:64) = padded xo grid with
   row stride 66; copy B (64:128) holds the next row's values at the same
   column (written directly by the dual-layout SE multiply).
 - conv3x3 = 6 dense K=128 matmuls per 16-row tile: M=128 packs (out
   channel x output-row-parity), K=128 packs (in channel x row shift).
   12288 PE rows total vs 24576 in the 9-tap formulation.
 - global-branch mean via DVE reduce over a [128, 2048] half-stacked copy
   of x (halves the reduce free size); stacked-G1 matmul recombines the
   partition halves exactly.
 - DMA: each HWDGE descriptor-gen costs a flat ~625ns and serializes, so
   bulk x_dual loads go through the Pool-engine SWDGE path instead, and
   everything else is batched into few transfers.
"""

import sys

if "/opt/trn_rl_repo" not in sys.path:
    sys.path.insert(0, "/opt/trn_rl_repo")

import numpy as np
import ml_dtypes

import concourse.bass as bass
import concourse.bacc as bacc
import concourse.mybir as mybir
import concourse.tile as tile
from concourse.bass_utils import run_bass_kernel_spmd

B, C, H, W = 8, 64, 64, 64
N = H * W
C2 = 2 * C          # 128
INTER = 16
EPS = 1e-5
PW = W + 2          # padded row stride = 66
BW = 2 * PW         # conv-view block width = 132 (one row pair)
NBLK = 40           # blocks in xo_pad; 40*132 = 5280 columns
PADC = NBLK * BW
HEAD = PW + 1       # flat offset of grid pixel (0, 0) = 67
CHUNK = 512
NCHUNK = N // CHUNK          # 8
ROWS_PER_CHUNK = CHUNK // W  # 8

TAPS = ((-1, -1), (-1, 0), (-1, 1), (1, -1), (1, 0), (1, 1))
TAPS0 = ((1, -1), (1, 0), (1, 1), (-1, -1), (-1, 0), (-1, 1))  # tile 0 order

F32 = mybir.dt.float32
BF16 = mybir.dt.bfloat16
AF = mybir.ActivationFunctionType
ALU = mybir.AluOpType
BFNP = ml_dtypes.bfloat16

# weight blob (bf16, 128 partitions) column layout
O_W1P = 0     # [128, 32]
O_W2P = 32    # [32, 128] on partitions 0:32
O_GW1P = 160  # [128, 16]
O_GW2P = 176  # [16, 128] on partitions 0:16
O_BIAS = 304  # 4 cols: b1 (0:32) | bsig | cb | gb1 (0:16)
WCOLS = 308

XH_SPLITS = ((0, 1024), (1024, 1536), (1536, 2048))
XD_SPLITS = ((0, 1024), (1024, 2048), (2048, 3072), (3072, 4096))

_prog_cache = {}


def _pix(r, w):
    """Flat column of valid grid pixel (r, w) in xo_pad copy A."""
    return HEAD + r * PW + w


def build_program(n_cores=8):
    nc = bacc.Bacc("TRN2", debug=False, target_bir_lowering=False,
                   num_devices=n_cores)

    wsm_d = nc.dram_tensor("wsm", [C2, WCOLS], BF16, kind="ExternalInput").ap()
    xh_d = nc.dram_tensor("xh", [C2, N // 2], BF16, kind="ExternalInput").ap()
    xd_d = nc.dram_tensor("xd", [C2, N], BF16, kind="ExternalInput").ap()
    cw6_d = nc.dram_tensor("cw6", [C2, 6 * C2], BF16,
                           kind="ExternalInput").ap()
    y_d = nc.dram_tensor("y", [C2, N // 2], BF16, kind="ExternalOutput").ap()

    with tile.TileContext(nc) as tc:
        with tc.tile_pool(name="big", bufs=1) as bpool, \
             tc.tile_pool(name="work", bufs=3) as wpool, \
             tc.tile_pool(name="t1s", bufs=8) as tpool, \
             tc.tile_pool(name="ps1p", bufs=2, space="PSUM") as pp1, \
             tc.tile_pool(name="ps2p", bufs=4, space="PSUM") as pp2, \
             tc.tile_pool(name="psyp", bufs=2, space="PSUM") as ppy:

            # dummy sigmoid at t~0: forces the single needed ACT table set
            # (sigmoid_and_others: sigmoid + relu + identity) to load early.
            scr = bpool.tile([1, 1], F32, tag="scr")
            nc.vector.memset(scr[:], 0)
            nc.scalar.activation(scr[:], scr[:], AF.Sigmoid)

            # ---- input DMAs.  SP queue: wsm, xh pieces, cw6 (HWDGE gen is
            # a serialized ~625ns per DMA).  Pool queue: xd pieces (SWDGE). --
            wsm = bpool.tile([C2, WCOLS], BF16, tag="wsm")
            nc.sync.dma_start(wsm[:], wsm_d)
            xh = bpool.tile([C2, N // 2], BF16, tag="xh")
            for lo, hi in XH_SPLITS:
                nc.sync.dma_start(xh[:, lo:hi], xh_d[:, lo:hi])
            xd = bpool.tile([C2, N], BF16, tag="xd")
            for lo, hi in XD_SPLITS:
                nc.gpsimd.dma_start(xd[:, lo:hi], xd_d[:, lo:hi])
            cw6 = bpool.tile([C2, 6 * C2], BF16, tag="cw6")
            nc.sync.dma_start(cw6[:], cw6_d)

            w1p = wsm[:, O_W1P:O_W1P + 32]
            w2p = wsm[0:32, O_W2P:O_W2P + C2]
            gw1p = wsm[:, O_GW1P:O_GW1P + INTER]
            gw2p = wsm[0:INTER, O_GW2P:O_GW2P + C2]

            # ---- xo_pad halo memsets (DVE dead time, before the reduces) ---
            xo_pad = bpool.tile([C2, PADC], BF16, tag="xopad")
            nc.vector.memset(xo_pad[:, 0:HEAD], 0)
            gaps = xo_pad[:, HEAD + W:HEAD + W + H * PW]
            gaps = gaps.rearrange("p (r w) -> p r w", w=PW)[:, :, 0:2]
            nc.vector.memset(gaps, 0)
            nc.vector.memset(xo_pad[:, _pix(H - 1, W) + 2:PADC], 0)
            # copy B's slot for grid row 64 (the bottom halo) stays zero
            nc.vector.memset(xo_pad[C:C2, _pix(H - 1, 0):_pix(H - 1, W)], 0)

            # ---- f32 bias columns, converted on-chip from the bf16 blob ----
            fbias = wpool.tile([C2, 4], F32, tag="fbias")
            nc.vector.tensor_copy(fbias[:], wsm[:, O_BIAS:O_BIAS + 4])
            b1 = fbias[0:32, 0:1]
            bsig = fbias[:, 1:2]
            cb = fbias[:, 2:3]
            gb1 = fbias[0:INTER, 3:4]

            # ---- global mean partials on DVE (from the half-stacked copy) --
            gparts = wpool.tile([C2, len(XH_SPLITS)], F32, tag="gparts")
            for q, (lo, hi) in enumerate(XH_SPLITS):
                nc.vector.reduce_sum(gparts[:, q:q + 1], xh[:, lo:hi],
                                     axis=mybir.AxisListType.X)
            g128 = wpool.tile([C2, 1], F32, tag="g128")
            nc.vector.reduce_sum(g128[:], gparts[:],
                                 axis=mybir.AxisListType.X)
            g128b = wpool.tile([C2, 1], BF16, tag="g128b")
            nc.vector.tensor_copy(g128b[:], g128[:])

            # ---- SE phase 1: mm1 + t1 relu for every chunk ----
            t1s = {}

            def emit_mm1(ci):
                ps1 = pp1.tile([32, CHUNK], F32, tag="ps1")
                nc.tensor.matmul(ps1[:], w1p,
                                 xd[:, ci * CHUNK:(ci + 1) * CHUNK],
                                 start=True, stop=True)
                t1 = tpool.tile([32, CHUNK], BF16, tag="t1")
                if ci < 5:
                    nc.scalar.activation(t1[:], ps1[:], AF.Relu, bias=b1)
                else:
                    nc.vector.tensor_scalar(t1[:], ps1[:], b1, 0.0,
                                            ALU.add, ALU.max)
                t1s[ci] = t1

            emit_mm1(0)
            emit_mm1(1)
            emit_mm1(2)
            emit_mm1(3)

            # ---- global branch MLP (PE ops land after mm1_3 in queue) ----
            psg1 = pp1.tile([INTER, 1], F32, tag="ps1")
            nc.tensor.matmul(psg1[:], gw1p, g128b[:], start=True, stop=True)
            g1 = wpool.tile([INTER, 1], BF16, tag="g1")
            nc.scalar.activation(g1[:], psg1[:], AF.Relu, bias=gb1,
                                 scale=1.0 / N)
            psg2 = pp2.tile([C2, 1], F32, tag="ps2")
            nc.tensor.matmul(psg2[:], gw2p, g1[:], start=True, stop=True)
            dbias = wpool.tile([C2, 1], F32, tag="dbias")
            nc.scalar.activation(dbias[:], psg2[:], AF.Identity, bias=bsig)

            for ci in range(4, NCHUNK):
                emit_mm1(ci)

            # ---- SE phase 2: mm2 + sigmoid + xo multiply ----
            def emit_mul(ci):
                sig = wpool.tile([C2, CHUNK], BF16, tag="sig")
                nc.scalar.activation(sig[:], ps2s[ci][:], AF.Sigmoid,
                                     bias=dbias[:])
                r0 = 8 * ci
                nrow = 8 if ci < NCHUNK - 1 else 7
                ncol = nrow * W
                dst = xo_pad[:, _pix(r0, 0):_pix(r0, 0) + nrow * PW]
                dst = dst.rearrange("p (r w) -> p r w", w=PW)[:, :, 0:W]
                xcr = xd[:, ci * CHUNK:ci * CHUNK + ncol]
                xcr = xcr.rearrange("p (r w) -> p r w", w=W)
                sgr = sig[:, 0:ncol].rearrange("p (r w) -> p r w", w=W)
                nc.vector.tensor_mul(dst, xcr, sgr)
                if ci == NCHUNK - 1:
                    # last row of the image: top half only (copy B's slot for
                    # it is the row-64 halo, which must stay zero)
                    dst2 = xo_pad[0:C, _pix(r0 + 7, 0):_pix(r0 + 7, 0) + W]
                    nc.vector.tensor_mul(
                        dst2, xd[0:C, ci * CHUNK + ncol:(ci + 1) * CHUNK],
                        sig[0:C, ncol:CHUNK])

            ps2s = {}
            for ci in range(NCHUNK):
                ps2 = pp2.tile([C2, CHUNK], F32, tag="ps2")
                nc.tensor.matmul(ps2[:], w2p, t1s[ci][:],
                                 start=True, stop=True)
                ps2s[ci] = ps2
                emit_mul(ci)

            # copy B's column for grid row -1 must hold xo row 0
            nc.sync.dma_start(xo_pad[C:C2, _pix(-1, 0):_pix(-1, 0) + W],
                              xo_pad[0:C, _pix(0, 0):_pix(0, 0) + W])

            # ---- conv3x3: 6 dense 128x128 matmuls per 16-row tile ----
            xor_v = xo_pad[:].rearrange("p (t w) -> p t w", w=BW)
            for k in range(4):
                r0 = 16 * k
                psy = ppy.tile([C2, CHUNK], F32, tag="psy")
                taps = TAPS0 if k == 0 else TAPS
                for i, (dlt, dx) in enumerate(taps):
                    jj = TAPS.index((dlt, dx))
                    t0 = (r0 + dlt + 1) // 2
                    rhs = xor_v[:, t0:t0 + 8, 1 + dx:1 + dx + W]
                    nc.tensor.matmul(psy[:], cw6[:, jj * C2:(jj + 1) * C2],
                                     rhs, start=(i == 0), stop=(i == 5))
                ybuf = wpool.tile([C2, CHUNK], BF16, tag="ybuf")
                nc.gpsimd.tensor_scalar(ybuf[:], psy[:], cb, 0.0,
                                        ALU.add, ALU.max)
                nc.sync.dma_start(y_d[:, k * CHUNK:(k + 1) * CHUNK], ybuf[:])

    nc.compile()
    return nc


def _affine(s, b, m, v):
    inv = s / np.sqrt(v + EPS)
    return inv, b - m * inv


def prepare_weights(inputs):
    f = lambda k: np.asarray(inputs[k], dtype=np.float32)
    a1, c1 = _affine(f("ls1"), f("lbb1"), f("lm1"), f("lv1"))
    W1 = a1[:, None] * f("lw1")              # [16, 64]
    B1 = a1 * f("lb1") + c1
    a2, c2 = _affine(f("ls2"), f("lbb2"), f("lm2"), f("lv2"))
    W2 = a2[:, None] * f("lw2")              # [64, 16]
    B2 = a2 * f("lb2") + c2
    ag1, cg1 = _affine(f("gs1"), f("gbb1"), f("gm1"), f("gv1"))
    G1 = ag1[:, None] * f("gw1")             # [16, 64]
    Bg1 = ag1 * f("gb1") + cg1
    ag2, cg2 = _affine(f("gs2"), f("gbb2"), f("gm2"), f("gv2"))
    G2 = ag2[:, None] * f("gw2")             # [64, 16]
    Bg2 = ag2 * f("gb2") + cg2
    ac, cc = _affine(f("cs"), f("cbb"), f("cm"), f("cv"))
    CW = ac[:, None, None, None] * f("cw")   # [64, 64, 3, 3] (o, c, ky, kx)
    CB = ac * f("cb") + cc

    w1p = np.zeros((C2, 32), np.float32)
    w1p[0:C, 0:INTER] = W1.T
    w1p[C:C2, INTER:32] = W1.T
    w2p = np.zeros((32, C2), np.float32)
    w2p[0:INTER, 0:C] = W2.T
    w2p[INTER:32, C:C2] = W2.T
    gw1p = np.concatenate([G1.T, G1.T], axis=0)        # [128, 16]
    gw2p = np.concatenate([G2.T, G2.T], axis=1)        # [16, 128]

    cw6 = np.zeros((C2, 6, C2), np.float32)
    for jj, (dlt, dx) in enumerate(TAPS):
        for s in (0, 1):
            for p in (0, 1):
                ky = dlt + s + 1 - p
                if 0 <= ky <= 2:
                    cw6[C * s:C * s + C, jj, C * p:C * p + C] = \
                        CW[:, :, ky, dx + 1].T

    wsm = np.zeros((C2, WCOLS), np.float32)
    wsm[:, O_W1P:O_W1P + 32] = w1p
    wsm[0:32, O_W2P:O_W2P + C2] = w2p
    wsm[:, O_GW1P:O_GW1P + INTER] = gw1p
    wsm[0:INTER, O_GW2P:O_GW2P + C2] = gw2p
    wsm[0:32, O_BIAS + 0] = np.concatenate([B1, B1])
    wsm[:, O_BIAS + 1] = np.concatenate([B2 + Bg2, B2 + Bg2])
    wsm[:, O_BIAS + 2] = np.concatenate([CB, CB])
    wsm[0:INTER, O_BIAS + 3] = Bg1
    return {
        "wsm": wsm.astype(BFNP),
        "cw6": np.ascontiguousarray(cw6.reshape(C2, 6 * C2)).astype(BFNP),
    }


def make_core_inputs(inputs):
    shared = prepare_weights(inputs)
    x = np.asarray(inputs["x"], dtype=np.float32)
    maps = []
    for i in range(B):
        xi = x[i]                                   # [64, 64, 64]
        xh = np.concatenate([xi[:, 0:H // 2, :].reshape(C, N // 2),
                             xi[:, H // 2:H, :].reshape(C, N // 2)], axis=0)
        xsh = np.concatenate([xi[:, 1:, :],
                              np.zeros((C, 1, W), np.float32)], axis=1)
        xd = np.concatenate([xi.reshape(C, N), xsh.reshape(C, N)], axis=0)
        maps.append({
            "wsm": shared["wsm"],
            "cw6": shared["cw6"],
            "xh": np.ascontiguousarray(xh).astype(BFNP),
            "xd": np.ascontiguousarray(xd).astype(BFNP),
        })
    return maps


def _unpack_y(y2):
    # y2 [128, 2048] bf16: [o, 512k + 64t + w] = y[o, 16k+2t, w];
    # partitions 64:128 hold the odd rows.
    y2 = np.asarray(y2, dtype=np.float32).reshape(2, C, 4, 8, W)
    out = np.empty((C, H, W), np.float32)
    r = np.arange(H)
    out = np.empty((C, 4, 8, 2, W), np.float32)
    out[:, :, :, 0, :] = y2[0]
    out[:, :, :, 1, :] = y2[1]
    return out.reshape(C, H, W)


def _run(inputs, trace=False):
    in_maps = make_core_inputs(inputs)
    if "prog" not in _prog_cache:
        _prog_cache["prog"] = build_program(B)
    nc = _prog_cache["prog"]
    res = run_bass_kernel_spmd(nc, in_maps, list(range(B)), trace=trace)
    out = np.stack([_unpack_y(r["y"]) for r in res.results])
    return out.astype(np.float32), res


def kernel(**inputs):
    out, _ = _run(inputs, trace=False)
    return out


def kernel_traced(inputs):
    return _run(inputs, trace=True)


def reference_numpy(inputs):
    """Pure-numpy emulation of the (dead-code-eliminated) reference using the
    same folded weights (f32, no bf16 rounding). Algebra validation only."""
    f = lambda k: np.asarray(inputs[k], dtype=np.float32)
    a1, c1 = _affine(f("ls1"), f("lbb1"), f("lm1"), f("lv1"))
    W1 = a1[:, None] * f("lw1")
    B1 = a1 * f("lb1") + c1
    a2, c2 = _affine(f("ls2"), f("lbb2"), f("lm2"), f("lv2"))
    W2 = a2[:, None] * f("lw2")
    B2 = a2 * f("lb2") + c2
    ag1, cg1 = _affine(f("gs1"), f("gbb1"), f("gm1"), f("gv1"))
    G1 = ag1[:, None] * f("gw1")
    Bg1 = ag1 * f("gb1") + cg1
    ag2, cg2 = _affine(f("gs2"), f("gbb2"), f("gm2"), f("gv2"))
    G2 = ag2[:, None] * f("gw2")
    Bg2 = ag2 * f("gb2") + cg2
    ac, cc = _affine(f("cs"), f("cbb"), f("cm"), f("cv"))
    CW = ac[:, None, None, None] * f("cw")
    CB = ac * f("cb") + cc
    x = np.asarray(inputs["x"], dtype=np.float32)
    out = np.empty_like(x)
    for i in range(B):
        xs = x[i].reshape(C, N)
        t1 = np.maximum(W1 @ xs + B1[:, None], 0.0)
        g = xs.mean(axis=1, keepdims=True)
        d = G2 @ np.maximum(G1 @ g + Bg1[:, None], 0.0) + (B2 + Bg2)[:, None]
        xo = xs / (1.0 + np.exp(-(W2 @ t1 + d)))
        xop = np.zeros((C, H + 2, W + 2), np.float32)
        xop[:, 1:-1, 1:-1] = xo.reshape(C, H, W)
        y = np.zeros((C, N), np.float32)
        for kk in range(9):
            ky, kx = divmod(kk, 3)
            sh = xop[:, ky:ky + H, kx:kx + W].reshape(C, N)
            y += CW[:, :, ky, kx] @ sh
        out[i] = np.maximum(y + CB[:, None], 0.0).reshape(C, H, W)
    return out


# revision 13
# speedup vs baseline: 1.6359x; 1.0424x over previous
"""Trainium2 Bass kernel for nn_Chan_spaAtt (SE-gated conv block), v3.

The spatial self-attention branch in the reference is dead code -- the output
depends only on xo = x * sigmoid(xl + xg) through the final 3x3 conv + BN +
ReLU (all BN affines folded host-side):

  t1   = relu(W1 @ x + b1)                      [16, N]
  d    = G2 @ relu(G1 @ mean(x) + bg1) + bsig   [64, 1]
  sarg = W2 @ t1                                [64, N]
  xo   = x * sigmoid(sarg + d)                  [64, N]
  y    = relu(conv3x3(xo, CW) + cb)             [64, N]

Sharding: one sample per NeuronCore (B=8).

Layout: everything bf16 on-chip, 128 partitions everywhere.
 - x_dual [128, 4096]: partition c+64s holds x[c, row+s] per 8-row chunk.
   The SE phase computes each pixel twice (once per shift) at zero extra
   cost: engine time scales with the free dim only.
 - xo_pad [128, 40*132]: copy A (partitions 0:64) = padded xo grid with
   row stride 66; copy B (64:128) holds the next row's values at the same
   column (written directly by the dual-layout SE multiply).
 - conv3x3 = 6 dense K=128 matmuls per 8-row half-tile: M=128 packs (out
   channel x output-row-parity), K=128 packs (in channel x row shift).
   12288 PE rows total vs 24576 in the 9-tap formulation.  The first
   half-tile replaces its three K=128 taps with K=64 pairs so it never
   reads copy B's unwritten row -1 column (no fixup DMA on the chain).
 - global-branch mean via DVE reduce over a [128, 2048] half-stacked copy
   of x; a stacked-G1 f32r matmul recombines the partition halves exactly.
 - DMA: HWDGE descriptor-gen costs a flat ~625ns serialized per transfer,
   so x_dual rides the Pool-engine SWDGE path and everything else is
   batched into few transfers, ordered so the mean-reduce stream lands
   first.
"""

import sys

if "/opt/trn_rl_repo" not in sys.path:
    sys.path.insert(0, "/opt/trn_rl_repo")

import numpy as np
import ml_dtypes

import concourse.bass as bass
import concourse.bacc as bacc
import concourse.mybir as mybir
import concourse.tile as tile
from concourse.bass_utils import run_bass_kernel_spmd

B, C, H, W = 8, 64, 64, 64
N = H * W
C2 = 2 * C          # 128
INTER = 16
EPS = 1e-5
PW = W + 2          # padded row stride = 66
BW = 2 * PW         # conv-view block width = 132 (one row pair)
NBLK = 40           # blocks in xo_pad; 40*132 = 5280 columns
PADC = NBLK * BW
HEAD = PW + 1       # flat offset of grid pixel (0, 0) = 67
CHUNK = 512
NCHUNK = N // CHUNK          # 8
HALF = 256                   # conv half-tile free size (4 row pairs)

TAPS = ((-1, -1), (-1, 0), (-1, 1), (1, -1), (1, 0), (1, 1))

F32 = mybir.dt.float32
F32R = mybir.dt.float32r
BF16 = mybir.dt.bfloat16
AF = mybir.ActivationFunctionType
ALU = mybir.AluOpType
BFNP = ml_dtypes.bfloat16

# weight blob (bf16, 128 partitions) column layout
O_W1P = 0      # [128, 32]
O_W2P = 32     # [32, 128] on partitions 0:32
O_GW1F = 160   # [128, 64] = [128, 32] f32 (bitcast), stacked G1
O_GW2P = 224   # [16, 128] on partitions 0:16
O_BIAS = 352   # 4 f32-as-2xbf16? no: 4 bf16 cols: b1 | bsig | cb | gb1
WCOLS = 356
# cw6 blob: 6 dense taps [128, 768] + cw3b (s=1 rows of the d=-1 taps,
# re-homed to partitions 0:64) [64, 384] at cols 768:1152
CW_COLS = 1152

XH_SPLITS = ((0, 1024), (1024, 2048))
XD_SPLITS = ((0, 1024), (1024, 2048), (2048, 4096))

_prog_cache = {}


def _pix(r, w):
    """Flat column of valid grid pixel (r, w) in xo_pad copy A."""
    return HEAD + r * PW + w


def build_program(n_cores=8):
    nc = bacc.Bacc("TRN2", debug=False, target_bir_lowering=False,
                   num_devices=n_cores)

    wsm_d = nc.dram_tensor("wsm", [C2, WCOLS], BF16, kind="ExternalInput").ap()
    xh_d = nc.dram_tensor("xh", [C2, N // 2], BF16, kind="ExternalInput").ap()
    xd_d = nc.dram_tensor("xd", [C2, N], BF16, kind="ExternalInput").ap()
    cw6_d = nc.dram_tensor("cw6", [C2, CW_COLS], BF16,
                           kind="ExternalInput").ap()
    y_d = nc.dram_tensor("y", [C2, N // 2], BF16, kind="ExternalOutput").ap()

    with tile.TileContext(nc) as tc:
        with tc.tile_pool(name="big", bufs=1) as bpool, \
             tc.tile_pool(name="work", bufs=3) as wpool, \
             tc.tile_pool(name="t1s", bufs=8) as tpool, \
             tc.tile_pool(name="ps1p", bufs=2, space="PSUM") as pp1, \
             tc.tile_pool(name="ps2p", bufs=3, space="PSUM") as pp2, \
             tc.tile_pool(name="psyp", bufs=3, space="PSUM") as ppy:

            # dummy sigmoid at t~0: forces the single needed ACT table set
            # (sigmoid_and_others: sigmoid + relu + identity) to load early.
            scr = bpool.tile([1, 1], F32, tag="scr")
            nc.vector.memset(scr[:], 0)
            nc.scalar.activation(scr[:], scr[:], AF.Sigmoid)

            # ---- input DMAs.  SP/HWDGE: mean-reduce stream first, then
            # weights.  Pool/SWDGE: x_dual pieces. ----
            xh = bpool.tile([C2, N // 2], BF16, tag="xh")
            for lo, hi in XH_SPLITS:
                nc.sync.dma_start(xh[:, lo:hi], xh_d[:, lo:hi])
            wsm = bpool.tile([C2, WCOLS], BF16, tag="wsm")
            nc.sync.dma_start(wsm[:], wsm_d)
            cw6 = bpool.tile([C2, CW_COLS], BF16, tag="cw6")
            nc.sync.dma_start(cw6[:], cw6_d)

            # ---- xo_pad halo memsets on Pool: they also delay the SWDGE
            # x_dual descriptor-gens just enough that the xh stream wins the
            # DMA-engine arbitration. ----
            xo_pad = bpool.tile([C2, PADC], BF16, tag="xopad")
            nc.gpsimd.memset(xo_pad[:, 0:HEAD], 0)
            gaps = xo_pad[:, HEAD + W:HEAD + W + H * PW]
            gaps = gaps.rearrange("p (r w) -> p r w", w=PW)[:, :, 0:2]
            nc.gpsimd.memset(gaps, 0)
            nc.gpsimd.memset(xo_pad[:, _pix(H - 1, W) + 2:PADC], 0)
            # copy B's slot for grid row 64 (the bottom halo) stays zero
            nc.gpsimd.memset(xo_pad[C:C2, _pix(H - 1, 0):_pix(H - 1, W)], 0)

            xd = bpool.tile([C2, N], BF16, tag="xd")
            for lo, hi in XD_SPLITS:
                nc.gpsimd.dma_start(xd[:, lo:hi], xd_d[:, lo:hi])

            w1p = wsm[:, O_W1P:O_W1P + 32]
            w2p = wsm[0:32, O_W2P:O_W2P + C2]
            gw1f = wsm[:, O_GW1F:O_GW1F + 32].bitcast(F32R)
            gw2p = wsm[0:INTER, O_GW2P:O_GW2P + C2]

            # ---- global mean partials on DVE (from the half-stacked copy) --
            gparts = wpool.tile([C2, len(XH_SPLITS)], F32, tag="gparts")
            for q, (lo, hi) in enumerate(XH_SPLITS):
                nc.vector.reduce_sum(gparts[:, q:q + 1], xh[:, lo:hi],
                                     axis=mybir.AxisListType.X)
            g128 = wpool.tile([C2, 1], F32, tag="g128")
            nc.vector.reduce_sum(g128[:], gparts[:],
                                 axis=mybir.AxisListType.X)

            # ---- f32 bias columns, converted on-chip from the bf16 blob ----
            fbias = wpool.tile([C2, 4], F32, tag="fbias")
            nc.vector.tensor_copy(fbias[:], wsm[:, O_BIAS:O_BIAS + 4])
            b1 = fbias[0:32, 0:1]
            bsig = fbias[:, 1:2]
            cb = fbias[:, 2:3]
            gb1 = fbias[0:INTER, 3:4]

            # ---- SE phase 1: mm1 for every chunk; t1 relu spread over
            # ACT (c0), Pool (odd), DVE (even, emitted inside phase 2) ----
            t1s = {}
            ps1s = {}

            def emit_mm1(ci):
                ps1 = pp1.tile([32, CHUNK], F32, tag="ps1")
                nc.tensor.matmul(ps1[:], w1p,
                                 xd[:, ci * CHUNK:(ci + 1) * CHUNK],
                                 start=True, stop=True)
                ps1s[ci] = ps1

            def emit_t1(ci, eng):
                t1 = tpool.tile([32, CHUNK], BF16, tag="t1")
                if eng == "act":
                    nc.scalar.activation(t1[:], ps1s[ci][:], AF.Relu, bias=b1)
                elif eng == "pool":
                    nc.gpsimd.tensor_scalar(t1[:], ps1s[ci][:], b1, 0.0,
                                            ALU.add, ALU.max)
                else:
                    nc.vector.tensor_scalar(t1[:], ps1s[ci][:], b1, 0.0,
                                            ALU.add, ALU.max)
                t1s[ci] = t1

            emit_mm1(0)
            emit_t1(0, "act")
            emit_mm1(1)
            emit_t1(1, "pool")
            emit_mm1(2)
            emit_mm1(3)
            emit_t1(3, "pool")

            # ---- global branch MLP (PE ops land after mm1_3 in queue) ----
            psg1 = ppy.tile([INTER, 1], F32, tag="psy")
            nc.tensor.matmul(psg1[:], gw1f, g128[:].bitcast(F32R),
                             start=True, stop=True)
            g1 = wpool.tile([INTER, 1], BF16, tag="g1")
            nc.scalar.activation(g1[:], psg1[:], AF.Relu, bias=gb1,
                                 scale=1.0 / N)
            psg2 = pp2.tile([C2, 1], F32, tag="ps2")
            nc.tensor.matmul(psg2[:], gw2p, g1[:], start=True, stop=True)
            dbias = wpool.tile([C2, 1], F32, tag="dbias")
            nc.scalar.activation(dbias[:], psg2[:], AF.Identity, bias=bsig)

            # ---- SE phase 2 + conv, software-pipelined ----
            def emit_mm2_sig(ci):
                ps2 = pp2.tile([C2, CHUNK], F32, tag="ps2")
                nc.tensor.matmul(ps2[:], w2p, t1s[ci][:],
                                 start=True, stop=True)
                sig = wpool.tile([C2, CHUNK], BF16, tag="sig")
                nc.scalar.activation(sig[:], ps2[:], AF.Sigmoid,
                                     bias=dbias[:])
                return sig

            def mul_rows(ci, sig, r0, nrow, top_only=False):
                pbase = C if top_only else C2
                off = (r0 - 8 * ci) * W
                dst = xo_pad[0:pbase, _pix(r0, 0):_pix(r0, 0) + nrow * PW]
                dst = dst.rearrange("p (r w) -> p r w", w=PW)[:, :, 0:W]
                xcr = xd[0:pbase, ci * CHUNK + off:ci * CHUNK + off + nrow * W]
                xcr = xcr.rearrange("p (r w) -> p r w", w=W)
                sgr = sig[0:pbase, off:off + nrow * W]
                sgr = sgr.rearrange("p (r w) -> p r w", w=W)
                nc.vector.tensor_mul(dst, xcr, sgr)

            def emit_mul(ci, sig):
                if ci < NCHUNK - 1:
                    mul_rows(ci, sig, 8 * ci, 8)
                else:
                    # split so conv h6 (needs only row 56) unblocks early;
                    # the bottom half's value for row 64 is never written
                    # (copy B's slot for it is the zero bottom halo)
                    mul_rows(ci, sig, 56, 1)
                    mul_rows(ci, sig, 57, 6)
                    mul_rows(ci, sig, 63, 1, top_only=True)

            # ---- conv3x3: 6 dense 128x128 matmuls per 8-row half-tile;
            # half 0 splits its d=-1 taps into K=64 pairs (no copy-B read
            # of the unwritten row -1 column). ----
            xor_v = xo_pad[:].rearrange("p (t w) -> p t w", w=BW)
            # shifted top-half view whose row-pair t holds content row 2t
            # (used by half 0 in place of copy B)
            xob_v = xo_pad[0:C, PW:PW + 4 * BW]
            xob_v = xob_v.rearrange("p (t w) -> p t w", w=BW)
            ysb = bpool.tile([C2, N // 2], BF16, tag="ysb")

            def emit_conv_half(j):
                psy = ppy.tile([C2, HALF], F32, tag="psy")
                first = True
                for dlt, dx in ((1, -1), (1, 0), (1, 1),
                                (-1, -1), (-1, 0), (-1, 1)):
                    jj = TAPS.index((dlt, dx))
                    wcol = jj * C2
                    t0 = (8 * j + dlt + 1) // 2
                    if j == 0 and dlt == -1:
                        nc.tensor.matmul(
                            psy[:], cw6[0:C, wcol:wcol + C2],
                            xor_v[0:C, t0:t0 + 4, 1 + dx:1 + dx + W],
                            start=False, stop=False)
                        kk = 768 + jj * C2
                        nc.tensor.matmul(
                            psy[:], cw6[0:C, kk:kk + C2],
                            xob_v[:, 0:4, 1 + dx:1 + dx + W],
                            start=False, stop=(dx == 1))
                    else:
                        nc.tensor.matmul(
                            psy[:], cw6[:, wcol:wcol + C2],
                            xor_v[:, t0:t0 + 4, 1 + dx:1 + dx + W],
                            start=first, stop=(j > 0 and dlt == -1
                                               and dx == 1))
                    first = False
                dsty = ysb[:, j * HALF:(j + 1) * HALF]
                if j < 4:
                    nc.gpsimd.tensor_scalar(dsty, psy[:], cb, 0.0,
                                            ALU.add, ALU.max)
                else:
                    nc.scalar.activation(dsty, psy[:], AF.Relu, bias=cb)

            sig0 = emit_mm2_sig(0)
            emit_mul(0, sig0)
            sig1 = emit_mm2_sig(1)
            emit_mul(1, sig1)
            emit_t1(2, "dve")
            sig2 = emit_mm2_sig(2)
            emit_mul(2, sig2)
            emit_mm1(4)
            emit_conv_half(0)
            emit_t1(4, "dve")
            sig3 = emit_mm2_sig(3)
            emit_mul(3, sig3)
            emit_mm1(5)
            emit_t1(5, "pool")
            emit_conv_half(1)
            sig4 = emit_mm2_sig(4)
            emit_mul(4, sig4)
            emit_mm1(6)
            emit_conv_half(2)
            emit_t1(6, "dve")
            sig5 = emit_mm2_sig(5)
            emit_mul(5, sig5)
            emit_mm1(7)
            emit_t1(7, "pool")
            emit_conv_half(3)
            sig6 = emit_mm2_sig(6)
            emit_mul(6, sig6)
            emit_conv_half(4)
            sig7 = emit_mm2_sig(7)
            emit_mul(7, sig7)
            emit_conv_half(5)
            nc.sync.dma_start(y_d[:, 0:4 * HALF], ysb[:, 0:4 * HALF])
            emit_conv_half(6)
            emit_conv_half(7)
            nc.sync.dma_start(y_d[:, 4 * HALF:7 * HALF],
                              ysb[:, 4 * HALF:7 * HALF])
            nc.sync.dma_start(y_d[:, 7 * HALF:8 * HALF],
                              ysb[:, 7 * HALF:8 * HALF])

    nc.compile()
    return nc


def _affine(s, b, m, v):
    inv = s / np.sqrt(v + EPS)
    return inv, b - m * inv


def prepare_weights(inputs):
    f = lambda k: np.asarray(inputs[k], dtype=np.float32)
    a1, c1 = _affine(f("ls1"), f("lbb1"), f("lm1"), f("lv1"))
    W1 = a1[:, None] * f("lw1")              # [16, 64]
    B1 = a1 * f("lb1") + c1
    a2, c2 = _affine(f("ls2"), f("lbb2"), f("lm2"), f("lv2"))
    W2 = a2[:, None] * f("lw2")              # [64, 16]
    B2 = a2 * f("lb2") + c2
    ag1, cg1 = _affine(f("gs1"), f("gbb1"), f("gm1"), f("gv1"))
    G1 = ag1[:, None] * f("gw1")             # [16, 64]
    Bg1 = ag1 * f("gb1") + cg1
    ag2, cg2 = _affine(f("gs2"), f("gbb2"), f("gm2"), f("gv2"))
    G2 = ag2[:, None] * f("gw2")             # [64, 16]
    Bg2 = ag2 * f("gb2") + cg2
    ac, cc = _affine(f("cs"), f("cbb"), f("cm"), f("cv"))
    CW = ac[:, None, None, None] * f("cw")   # [64, 64, 3, 3] (o, c, ky, kx)
    CB = ac * f("cb") + cc

    w1p = np.zeros((C2, 32), np.float32)
    w1p[0:C, 0:INTER] = W1.T
    w1p[C:C2, INTER:32] = W1.T
    w2p = np.zeros((32, C2), np.float32)
    w2p[0:INTER, 0:C] = W2.T
    w2p[INTER:32, C:C2] = W2.T
    gw1f = np.concatenate([G1.T, G1.T], axis=0).astype(np.float32)  # [128,16]
    gw2p = np.concatenate([G2.T, G2.T], axis=1)                     # [16,128]

    cw6 = np.zeros((C2, 6, C2), np.float32)
    for jj, (dlt, dx) in enumerate(TAPS):
        for s in (0, 1):
            for p in (0, 1):
                ky = dlt + s + 1 - p
                if 0 <= ky <= 2:
                    cw6[C * s:C * s + C, jj, C * p:C * p + C] = \
                        CW[:, :, ky, dx + 1].T
    # s=1 rows of the d=-1 taps, re-homed to partitions 0:64
    cw3b = np.zeros((C2, 3, C2), np.float32)
    for jj in range(3):
        cw3b[0:C, jj, :] = cw6[C:C2, jj, :]

    wsm = np.zeros((C2, WCOLS), np.float32)
    wsm[:, O_W1P:O_W1P + 32] = w1p
    wsm[0:32, O_W2P:O_W2P + C2] = w2p
    wsm[0:INTER, O_GW2P:O_GW2P + C2] = gw2p
    wsm[0:32, O_BIAS + 0] = np.concatenate([B1, B1])
    wsm[:, O_BIAS + 1] = np.concatenate([B2 + Bg2, B2 + Bg2])
    wsm[:, O_BIAS + 2] = np.concatenate([CB, CB])
    wsm[0:INTER, O_BIAS + 3] = Bg1
    wsm16 = wsm.astype(BFNP)
    # G1 kept in f32 (as bf16 bit-pairs) for the f32r mean matmul
    wsm16[:, O_GW1F:O_GW1F + 32] = np.ascontiguousarray(gw1f).view(BFNP)

    cwblob = np.zeros((C2, CW_COLS), np.float32)
    cwblob[:, 0:768] = cw6.reshape(C2, 768)
    cwblob[:, 768:1152] = cw3b.reshape(C2, 384)
    return {
        "wsm": wsm16,
        "cw6": cwblob.astype(BFNP),
    }


def make_core_inputs(inputs):
    shared = prepare_weights(inputs)
    x = np.asarray(inputs["x"], dtype=np.float32)
    maps = []
    for i in range(B):
        xi = x[i]                                   # [64, 64, 64]
        xh = np.concatenate([xi[:, 0:H // 2, :].reshape(C, N // 2),
                             xi[:, H // 2:H, :].reshape(C, N // 2)], axis=0)
        xsh = np.concatenate([xi[:, 1:, :],
                              np.zeros((C, 1, W), np.float32)], axis=1)
        xd = np.concatenate([xi.reshape(C, N), xsh.reshape(C, N)], axis=0)
        maps.append({
            "wsm": shared["wsm"],
            "cw6": shared["cw6"],
            "xh": np.ascontiguousarray(xh).astype(BFNP),
            "xd": np.ascontiguousarray(xd).astype(BFNP),
        })
    return maps


def _unpack_y(y2):
    # y2 [128, 2048] bf16: [o, 256j + 64t + w] = y[o, 8j+2t, w];
    # partitions 64:128 hold the odd rows.
    y2 = np.asarray(y2, dtype=np.float32).reshape(2, C, 8, 4, W)
    out = np.empty((C, 8, 4, 2, W), np.float32)
    out[:, :, :, 0, :] = y2[0]
    out[:, :, :, 1, :] = y2[1]
    return out.reshape(C, H, W)


def _run(inputs, trace=False):
    in_maps = make_core_inputs(inputs)
    if "prog" not in _prog_cache:
        _prog_cache["prog"] = build_program(B)
    nc = _prog_cache["prog"]
    res = run_bass_kernel_spmd(nc, in_maps, list(range(B)), trace=trace)
    out = np.stack([_unpack_y(r["y"]) for r in res.results])
    return out.astype(np.float32), res


def kernel(**inputs):
    out, _ = _run(inputs, trace=False)
    return out


def kernel_traced(inputs):
    return _run(inputs, trace=True)


def reference_numpy(inputs):
    """Pure-numpy emulation of the (dead-code-eliminated) reference using the
    same folded weights (f32, no bf16 rounding). Algebra validation only."""
    f = lambda k: np.asarray(inputs[k], dtype=np.float32)
    a1, c1 = _affine(f("ls1"), f("lbb1"), f("lm1"), f("lv1"))
    W1 = a1[:, None] * f("lw1")
    B1 = a1 * f("lb1") + c1
    a2, c2 = _affine(f("ls2"), f("lbb2"), f("lm2"), f("lv2"))
    W2 = a2[:, None] * f("lw2")
    B2 = a2 * f("lb2") + c2
    ag1, cg1 = _affine(f("gs1"), f("gbb1"), f("gm1"), f("gv1"))
    G1 = ag1[:, None] * f("gw1")
    Bg1 = ag1 * f("gb1") + cg1
    ag2, cg2 = _affine(f("gs2"), f("gbb2"), f("gm2"), f("gv2"))
    G2 = ag2[:, None] * f("gw2")
    Bg2 = ag2 * f("gb2") + cg2
    ac, cc = _affine(f("cs"), f("cbb"), f("cm"), f("cv"))
    CW = ac[:, None, None, None] * f("cw")
    CB = ac * f("cb") + cc
    x = np.asarray(inputs["x"], dtype=np.float32)
    out = np.empty_like(x)
    for i in range(B):
        xs = x[i].reshape(C, N)
        t1 = np.maximum(W1 @ xs + B1[:, None], 0.0)
        g = xs.mean(axis=1, keepdims=True)
        d = G2 @ np.maximum(G1 @ g + Bg1[:, None], 0.0) + (B2 + Bg2)[:, None]
        xo = xs / (1.0 + np.exp(-(W2 @ t1 + d)))
        xop = np.zeros((C, H + 2, W + 2), np.float32)
        xop[:, 1:-1, 1:-1] = xo.reshape(C, H, W)
        y = np.zeros((C, N), np.float32)
        for kk in range(9):
            ky, kx = divmod(kk, 3)
            sh = xop[:, ky:ky + H, kx:kx + W].reshape(C, N)
            y += CW[:, :, ky, kx] @ sh
        out[i] = np.maximum(y + CB[:, None], 0.0).reshape(C, H, W)
    return out
